# revision 1
# baseline (speedup 1.0000x reference)
"""Trainium2 Bass kernel for nn_CorrelationLayer: B=16, C=256, H=W=128, max_disp=4.

out[b, (dy+4)*9+(dx+4), y, x] = sum_c f1n[b,c,y,x] * f2n[b,c,y-dy,x-dx]
with f1n/f2n L2-normalized over c (zero-fill outside the image).

Strategy (per core, 2 samples, pure batch data-parallel over 8 cores):
  For each 8-row band of the image, for each 8x16-pixel patch:
    - PE computes a banded Gram: lhsT = f1 patch [C, 128 px] (float32r),
      rhs = f2 window rows y0-4..y0+11 x cols x0-4..x0+19 held in a 16-row
      ring buffer [C, 16*24] -> PSUM G[128, 384], K=256 via 2-pass accum.
    - DVE fuses the normalization: data = G * (1/||f1[px]||) * (1/||f2[win]||)
      (row scalar per partition, column vector broadcast over partitions).
    - GPSIMD indirect_copy realigns the 9 needed window rows per pixel-row r
      (indices shared within each 16-partition group == one r)
      -> H[128 px, 9*24] in (a=4-dy, e=cx+4-dx) coordinates.
    - PE transposes H -> [216 planes, 128 px]; ACT copies into band tiles;
      DMA stores to a padded output [216, H, W] with 512B runs.
  Host unscrambles e -> dx (x%16-dependent slice) and reverses both disp axes.
  Norms: ACT squares (float32r) + PE ones-matmul partition-reduction
  + ACT sqrt + DVE reciprocal.
"""

import sys

sys.path.insert(0, "/opt/trn_rl_repo")

import numpy as np

B, C, H, W = 16, 256, 128, 128
NCORES = 8
SPC = B // NCORES        # samples per core
CHI = 2                  # c = chi*128 + partition
NB = 16                  # bands per image
RB = 8                   # rows per band
NP = 8                   # patches per band
PW = 16                  # patch width
WIN = 24                 # f2 window width per patch (16 + 8)
SLOTS = 16               # ring rows
RW = 136                 # ring width (128 + 2*4 zero pad)
NA = 9                   # row displacements (a = 4 - dy)
NEB = 3                  # 8-element blocks per window row
INNER = 8
NIDX = 28                # 27 used + 1 pad (alignment)
HFREE = NIDX * INNER     # 224
USED = NA * WIN          # 216
GN = SLOTS * WIN         # gram free size 384

_CACHE = {}


def _idx_table(parity: int) -> np.ndarray:
    t = np.zeros((128, 2), dtype=np.uint16)
    for r in range(RB):
        for j in range(NA * NEB):
            a, eb = divmod(j, NEB)
            slot = (8 * parity + r + a + 12) % SLOTS
            t[16 * r + (j % 16), j // 16] = slot * WIN + INNER * eb
    return t


def _build_nc():
    import concourse.tile as tile
    from concourse import bacc, mybir

    f32 = mybir.dt.float32
    f32r = mybir.dt.float32r

    nc = bacc.Bacc("TRN2", target_bir_lowering=False, debug=False)

    f1d = nc.dram_tensor("feat1", [SPC, C, H, W], f32r, kind="ExternalInput").ap()
    f2d = nc.dram_tensor("feat2", [SPC, C, H, W], f32r, kind="ExternalInput").ap()
    idx_e = nc.dram_tensor("idx_e", [128, 2], mybir.dt.uint16, kind="ExternalInput").ap()
    idx_o = nc.dram_tensor("idx_o", [128, 2], mybir.dt.uint16, kind="ExternalInput").ap()
    ident = nc.dram_tensor("ident", [128, 128], f32, kind="ExternalInput").ap()
    ones = nc.dram_tensor("ones", [128, 1], mybir.dt.float16, kind="ExternalInput").ap()
    zeros = nc.dram_tensor("zeros", [128, 1088], f32r, kind="ExternalInput").ap()
    opad = nc.dram_tensor("out_pad", [SPC, USED, H, W], f32, kind="ExternalOutput").ap()

    f1v = f1d.rearrange("s (chi p) y x -> s p chi y x", p=128)
    f2v = f2d.rearrange("s (chi p) y x -> s p chi y x", p=128)

    with tile.TileContext(nc) as tc:
        with (
            tc.tile_pool(name="persist", bufs=1) as pers,
            tc.tile_pool(name="f1p", bufs=2) as f1p,
            tc.tile_pool(name="sq1p", bufs=2) as sq1p,
            tc.tile_pool(name="sq2p", bufs=2) as sq2p,
            tc.tile_pool(name="s2tp", bufs=2) as s2tp,
            tc.tile_pool(name="s1p", bufs=2) as s1p,
            tc.tile_pool(name="datap", bufs=3) as datap,
            tc.tile_pool(name="hp", bufs=3) as hp,
            tc.tile_pool(name="htp", bufs=2) as htp,
            tc.tile_pool(name="gps", bufs=2, space="PSUM") as gps,
            tc.tile_pool(name="tps", bufs=2, space="PSUM") as tps,
            tc.tile_pool(name="s1ps", bufs=2, space="PSUM") as s1ps,
            tc.tile_pool(name="s2ps", bufs=2, space="PSUM") as s2ps,
        ):
            # persistent tiles
            ring = pers.tile([128, CHI, SLOTS, RW], f32r)      # f2 rows
            s2ring = pers.tile([1, SLOTS, RW], f32)            # 1/||f2|| rows
            s2ringB = pers.tile([128, SLOTS, RW], f32)         # bcast over partitions
            isbe = pers.tile([128, 2], mybir.dt.uint16)
            isbo = pers.tile([128, 2], mybir.dt.uint16)
            idsb = pers.tile([128, 128], f32)
            onesb = pers.tile([128, 1], mybir.dt.float16)

            nc.sync.dma_start(isbe[:], idx_e)
            nc.sync.dma_start(isbo[:], idx_o)
            nc.sync.dma_start(idsb[:], ident)
            nc.sync.dma_start(onesb[:], ones)

            # zero x-pad columns of the rings (once; loads never touch them)
            zv = zeros.rearrange("p (a b) -> p a b", b=4)
            for chi in range(CHI):
                nc.sync.dma_start(ring[:, chi, :, 0:4], zv[:, 0:SLOTS, :])
                nc.sync.dma_start(ring[:, chi, :, 132:136], zv[:, 0:SLOTS, :])
            nc.vector.memset(s2ring[:], 0.0)
            nc.vector.memset(s2ringB[:], 0.0)

            def load_f2_rows(s, rows):
                """DMA f2 rows (list of consecutive image rows) into ring slots,
                plus squares -> norms -> s2ring. rows are 4-aligned groups."""
                # group by consecutive slots
                i = 0
                while i < len(rows):
                    j = i
                    while (
                        j + 1 < len(rows)
                        and rows[j + 1] % SLOTS == (rows[j] % SLOTS) + 1
                    ):
                        j += 1
                    run = rows[i : j + 1]
                    s0 = run[0] % SLOTS
                    n = len(run)
                    for chi in range(CHI):
                        nc.sync.dma_start(
                            ring[:, chi, s0 : s0 + n, 4 : 4 + W],
                            f2v[s, :, chi, run[0] : run[0] + n, :],
                        )
                    i = j + 1
                # norms in 4-row chunks
                for i in range(0, len(rows), 4):
                    chunk = rows[i : i + 4]
                    s0 = chunk[0] % SLOTS
                    ncr = len(chunk)
                    sq2 = sq2p.tile([128, CHI, 4, W], mybir.dt.float16, tag="sq2")
                    nc.scalar.activation(
                        sq2[:, :, :ncr, :],
                        ring[:, :, s0 : s0 + ncr, 4 : 4 + W].bitcast(f32),
                        mybir.ActivationFunctionType.Square,
                    )
                    ss = s2ps.tile([1, 4 * W], f32, tag="s2ss")
                    for chi in range(CHI):
                        nc.tensor.matmul(
                            ss[:, : ncr * W],
                            onesb[:],
                            sq2[:, chi, :ncr, :],
                            start=(chi == 0),
                            stop=(chi == CHI - 1),
                        )
                    s2t = s2tp.tile([1, 4 * W], f32, tag="s2t")
                    nc.scalar.activation(
                        s2t[:, : ncr * W],
                        ss[:, : ncr * W],
                        mybir.ActivationFunctionType.Sqrt,
                    )
                    nc.vector.reciprocal(
                        s2ring[:, s0 : s0 + ncr, 4 : 4 + W],
                        s2t[:, : ncr * W].rearrange("p (a b) -> p a b", b=W),
                    )
                    nc.gpsimd.partition_broadcast(
                        s2ringB[:, s0 : s0 + ncr, :],
                        s2ring[0:1, s0 : s0 + ncr, :],
                    )

            def zero_slots(slots):
                s0, n = slots[0], len(slots)
                zv2 = zeros.rearrange("p (a b) -> p a b", b=RW)
                for chi in range(CHI):
                    nc.sync.dma_start(ring[:, chi, s0 : s0 + n, :], zv2[:, 0:n, :])

            for s in range(SPC):
                for b in range(NB):
                    y0 = RB * b
                    isb = isbe if b % 2 == 0 else isbo

                    # --- f2 ring maintenance: ensure rows [y0-4, y0+12) ---
                    if b == 0:
                        zero_slots([12, 13, 14, 15])  # rows -4..-1
                        load_f2_rows(s, list(range(0, 12)))
                    elif b == NB - 1:
                        load_f2_rows(s, list(range(y0 + 4, H)))
                        zero_slots([0, 1, 2, 3])      # rows 128..131
                    else:
                        load_f2_rows(s, list(range(y0 + 4, y0 + 12)))

                    # --- f1 band + norms (patch-major: [chi, patch, r, cx]) ---
                    f1t = f1p.tile([128, CHI, NP, RB, PW], f32r, tag="f1")
                    for chi in range(CHI):
                        for r in range(RB):
                            nc.sync.dma_start(
                                f1t[:, chi, :, r, :],
                                f1v[s, :, chi, y0 + r, :].rearrange(
                                    "p (np pw) -> p np pw", pw=PW
                                ),
                            )
                    sq1 = sq1p.tile([128, CHI, NP, RB, PW], mybir.dt.float16, tag="sq1")
                    nc.scalar.activation(
                        sq1[:],
                        f1t[:].bitcast(f32),
                        mybir.ActivationFunctionType.Square,
                    )
                    s1ss = s1ps.tile([128, NP], f32, tag="s1ss")
                    for p in range(NP):
                        for chi in range(CHI):
                            nc.tensor.matmul(
                                s1ss[:, p : p + 1],
                                sq1[:, chi, p].rearrange("p a b -> p (a b)"),
                                onesb[:],
                                start=(chi == 0),
                                stop=(chi == CHI - 1),
                            )
                    s1n = s1p.tile([128, NP], f32, tag="s1n")
                    nc.scalar.activation(
                        s1n[:], s1ss[:], mybir.ActivationFunctionType.Sqrt
                    )
                    s1r = s1p.tile([128, NP], f32, tag="s1r")
                    nc.vector.reciprocal(s1r[:], s1n[:])

                    # --- output band tiles ---
                    ht0 = htp.tile([108, RB, NP, PW], f32, tag="ht0")
                    ht1 = htp.tile([108, RB, NP, PW], f32, tag="ht1")

                    for p in range(NP):
                        x0 = PW * p
                        # Gram
                        g = gps.tile([128, SLOTS, WIN], f32, tag="g")
                        for chi in range(CHI):
                            nc.tensor.matmul(
                                g[:].rearrange("p s x -> p (s x)"),
                                f1t[:, chi, p].rearrange("p a b -> p (a b)"),
                                ring[:, chi, :, x0 : x0 + WIN],
                                start=(chi == 0),
                                stop=(chi == CHI - 1),
                            )
                        # fused normalization
                        data = datap.tile([128, GN], f32, tag="data")
                        nc.vector.scalar_tensor_tensor(
                            data[:].rearrange("p (a b) -> p a b", b=WIN),
                            g[:],
                            s1r[:, p : p + 1],
                            s2ringB[:, :, x0 : x0 + WIN],
                            op0=mybir.AluOpType.mult,
                            op1=mybir.AluOpType.mult,
                        )
                        # realign window rows per pixel-row (shared idx per r)
                        ht = hp.tile([128, HFREE], f32, tag="h")
                        nc.gpsimd.indirect_copy(
                            ht[:].rearrange("p (a b) -> p a b", b=INNER),
                            data[:].rearrange("p (a b) -> p a b", b=INNER),
                            isb[:],
                            True,
                        )
                        # transpose the 216 used planes
                        tp = tps.tile([108, 256], f32, tag="t")
                        nc.tensor.transpose(tp[:, 0:128], ht[:, 0:108], idsb[:])
                        nc.tensor.transpose(tp[:, 128:256], ht[:, 108:216], idsb[:])
                        nc.scalar.copy(ht0[:, :, p, :], tp[:, 0:128].rearrange(
                            "d (r cx) -> d r cx", cx=PW))
                        nc.scalar.copy(ht1[:, :, p, :], tp[:, 128:256].rearrange(
                            "d (r cx) -> d r cx", cx=PW))

                    opv = opad[s, :, y0 : y0 + RB, :].rearrange(
                        "d r (p cx) -> d r p cx", cx=PW
                    )
                    nc.scalar.dma_start(opv[0:108], ht0[:])
                    nc.scalar.dma_start(opv[108:216], ht1[:])

    nc.compile()
    return nc


def _constants():
    return {
        "idx_e": _idx_table(0),
        "idx_o": _idx_table(1),
        "ident": np.eye(128, dtype=np.float32),
        "ones": np.ones((128, 1), dtype=np.float16),
        "zeros": np.zeros((128, 1088), dtype=np.float32),
    }


def _unscramble(pad: np.ndarray) -> np.ndarray:
    """pad [SPC, 216, H, W] -> out [SPC, 81, H, W].

    pad[s, a*24+e, y, x] = corr for dy = 4-a, dx = 4-(e - x%16)."""
    pad = pad.reshape(SPC, NA, WIN, H, W)
    tmp = np.empty((SPC, NA, NA, H, W), dtype=pad.dtype)
    for xm in range(PW):
        tmp[:, :, :, :, xm::PW] = pad[:, :, xm : xm + NA, :, xm::PW]
    # d order: (dy+4, dx+4) = (8-a, 8-adx)
    return tmp[:, ::-1, ::-1].reshape(SPC, NA * NA, H, W)


def kernel(feat1: np.ndarray, feat2: np.ndarray) -> np.ndarray:
    from concourse.bass_utils import run_bass_kernel_spmd

    feat1 = np.ascontiguousarray(np.asarray(feat1, dtype=np.float32))
    feat2 = np.ascontiguousarray(np.asarray(feat2, dtype=np.float32))

    if "nc" not in _CACHE:
        _CACHE["nc"] = _build_nc()
    nc = _CACHE["nc"]

    consts = _constants()
    in_maps = []
    for core in range(NCORES):
        s0 = SPC * core
        m = {"feat1": feat1[s0 : s0 + SPC], "feat2": feat2[s0 : s0 + SPC]}
        m.update(consts)
        in_maps.append(m)

    trace = _CACHE.get("trace", False)
    res = run_bass_kernel_spmd(nc, in_maps, list(range(NCORES)), trace=trace)
    _CACHE["last_results"] = res

    out = np.empty((B, NA * NA, H, W), dtype=np.float32)
    for core in range(NCORES):
        s0 = SPC * core
        out[s0 : s0 + SPC] = _unscramble(res.results[core]["out_pad"])
    return out



# revision 7
# speedup vs baseline: 1.1551x; 1.1551x over previous
"""Trainium2 Bass kernel for nn_CorrelationLayer: B=16, C=256, H=W=128, max_disp=4.

out[b, (dy+4)*9+(dx+4), y, x] = sum_c f1n[b,c,y,x] * f2n[b,c,y-dy,x-dx]
with f1n/f2n L2-normalized over c (zero-fill outside the image).

Strategy (per core, 2 samples, pure batch data-parallel over 8 cores):
  Bands (8 rows) x patches (8x16 px). Per patch the PE computes a banded
  Gram G[128 px, 16 slots * 24 wx] against a 16-row f2 ring buffer
  (K=256 via 2-pass PSUM accumulation, f32r operands).
  DVE fuses normalization: data = G * rsqrt(|f1[px]|^2) * rsqrt(|f2[win]|^2)
  (row scalar per partition + column vector broadcast). rsqrt is computed
  on ACT as exp(-0.5*ln(x)) to avoid the 1-lane DVE reciprocal.
  GPSIMD indirect_copy realigns window rows per pixel-row -> H[128, 9*24].
  PE transposes H via a truncated-bf16 bitcast view (1 cyc/row) ->
  [216 planes, 128 px]; ACT+DVE copy halves into bf16 band tiles; DMA
  stores to a padded bf16 output [216, H, W] (2KB runs). Host converts to
  f32 and unscrambles e -> dx, reversing both disp axes.

  The two samples of a core are processed interleaved (independent rings)
  so each sample's f2 ring refill + norm chain overlaps the other
  sample's compute instead of stalling on ring reuse.
"""

import sys

sys.path.insert(0, "/opt/trn_rl_repo")

import numpy as np

B, C, H, W = 16, 256, 128, 128
NCORES = 8
SPC = B // NCORES        # samples per core
CHI = 2                  # c = chi*128 + partition
NB = 16                  # bands per image
RB = 8                   # rows per band
NP = 8                   # patches per band
PW = 16                  # patch width
WIN = 24                 # f2 window width per patch (16 + 8)
SLOTS = 16               # ring rows
RW = 136                 # ring width (128 + 2*4 zero pad)
NA = 9                   # row displacements (a = 4 - dy)
NEB = 3                  # 8-element blocks per window row
INNER = 8
NIDX = 28                # 27 used + 1 pad (alignment)
HFREE = NIDX * INNER     # 224
USED = NA * WIN          # 216
GN = SLOTS * WIN         # gram free size 384

_CACHE = {}


def _idx_table(parity: int) -> np.ndarray:
    t = np.zeros((128, 2), dtype=np.uint16)
    for r in range(RB):
        for j in range(NA * NEB):
            a, eb = divmod(j, NEB)
            slot = (8 * parity + r + a + 12) % SLOTS
            t[16 * r + (j % 16), j // 16] = slot * WIN + INNER * eb
    return t


def _build_nc():
    import concourse.tile as tile
    from concourse import bacc, mybir

    f32 = mybir.dt.float32
    f32r = mybir.dt.float32r
    bf16 = mybir.dt.bfloat16

    nc = bacc.Bacc("TRN2", target_bir_lowering=False, debug=False)

    f1d = nc.dram_tensor("feat1", [SPC, C, H, W], f32r, kind="ExternalInput").ap()
    f2d = nc.dram_tensor("feat2", [SPC, C, H, W], f32r, kind="ExternalInput").ap()
    idx_e = nc.dram_tensor("idx_e", [128, 2], mybir.dt.uint16, kind="ExternalInput").ap()
    idx_o = nc.dram_tensor("idx_o", [128, 2], mybir.dt.uint16, kind="ExternalInput").ap()
    ident = nc.dram_tensor("ident", [128, 128], bf16, kind="ExternalInput").ap()
    ones = nc.dram_tensor("ones", [128, 1], mybir.dt.float16, kind="ExternalInput").ap()
    zeros = nc.dram_tensor("zeros", [128, 1088], f32r, kind="ExternalInput").ap()
    opad = nc.dram_tensor("out_pad", [SPC, USED, H, W], bf16, kind="ExternalOutput").ap()

    f1v = f1d.rearrange("s (chi p) y x -> s p chi y x", p=128)
    f2v = f2d.rearrange("s (chi p) y x -> s p chi y x", p=128)

    def hi_bf16(ap):
        """Truncated-bf16 view of an f32 AP (high half of each word)."""
        n = ap.shape[-1]
        return ap.bitcast(bf16).rearrange(
            "p (n two) -> p n two", two=2
        )[:, :, 1]

    with tile.TileContext(nc) as tc:
        with (
            tc.tile_pool(name="persist", bufs=1) as pers,
            tc.tile_pool(name="f1p", bufs=2) as f1p,
            tc.tile_pool(name="f1pmp", bufs=2) as f1pmp,
            tc.tile_pool(name="sq1p", bufs=2) as sq1p,
            tc.tile_pool(name="sq2p", bufs=2) as sq2p,
            tc.tile_pool(name="s2tp", bufs=2) as s2tp,
            tc.tile_pool(name="s1p", bufs=2) as s1p,
            tc.tile_pool(name="datap", bufs=3) as datap,
            tc.tile_pool(name="hp", bufs=3) as hp,
            tc.tile_pool(name="htp", bufs=2) as htp,
            tc.tile_pool(name="gps", bufs=2, space="PSUM") as gps,
            tc.tile_pool(name="tps", bufs=2, space="PSUM") as tps,
            tc.tile_pool(name="s1ps", bufs=2, space="PSUM") as s1ps,
            tc.tile_pool(name="s2ps", bufs=2, space="PSUM") as s2ps,
        ):
            # persistent tiles (per-sample rings)
            ring = pers.tile([128, SPC, CHI, SLOTS, RW], f32r)   # f2 rows
            s2g = pers.tile([1, SPC, SLOTS, RW], f32)            # rsqrt(|f2 row|^2)
            s2gB = pers.tile([128, SPC, SLOTS, RW], f32)         # bcast over partitions
            isbe = pers.tile([128, 2], mybir.dt.uint16)
            isbo = pers.tile([128, 2], mybir.dt.uint16)
            idsb = pers.tile([128, 128], bf16)
            onesb = pers.tile([128, 1], mybir.dt.float16)

            nc.sync.dma_start(isbe[:], idx_e)
            nc.sync.dma_start(isbo[:], idx_o)
            nc.sync.dma_start(idsb[:], ident)
            nc.sync.dma_start(onesb[:], ones)

            # zero x-pad columns of the rings (once; loads never touch them)
            zv = zeros.rearrange("p (a b) -> p a b", b=4)
            for s in range(SPC):
                for chi in range(CHI):
                    nc.sync.dma_start(ring[:, s, chi, :, 0:4], zv[:, 0:SLOTS, :])
                    nc.sync.dma_start(ring[:, s, chi, :, 132:136], zv[:, 0:SLOTS, :])
            nc.vector.memset(s2g[:], 0.0)
            nc.vector.memset(s2gB[:], 0.0)

            def load_f2_rows(s, rows):
                """DMA f2 rows (consecutive image rows) into ring slots, plus
                squares -> norms -> rsqrt -> broadcast. rows are 4-aligned."""
                i = 0
                while i < len(rows):
                    j = i
                    while (
                        j + 1 < len(rows)
                        and rows[j + 1] % SLOTS == (rows[j] % SLOTS) + 1
                    ):
                        j += 1
                    run = rows[i : j + 1]
                    s0 = run[0] % SLOTS
                    n = len(run)
                    for chi in range(CHI):
                        nc.sync.dma_start(
                            ring[:, s, chi, s0 : s0 + n, 4 : 4 + W],
                            f2v[s, :, chi, run[0] : run[0] + n, :],
                        )
                    i = j + 1
                # norms in 4-row chunks: rsqrt(ss) = exp(-0.5 * ln(ss))
                for i in range(0, len(rows), 4):
                    chunk = rows[i : i + 4]
                    s0 = chunk[0] % SLOTS
                    ncr = len(chunk)
                    sq2 = sq2p.tile([128, CHI, 4, W], mybir.dt.float16, tag="sq2")
                    nc.scalar.activation(
                        sq2[:, :, :ncr, :],
                        ring[:, s, :, s0 : s0 + ncr, 4 : 4 + W].bitcast(f32),
                        mybir.ActivationFunctionType.Square,
                    )
                    ss = s2ps.tile([1, 4 * W], f32, tag="s2ss")
                    for chi in range(CHI):
                        nc.tensor.matmul(
                            ss[:, : ncr * W],
                            onesb[:],
                            sq2[:, chi, :ncr, :],
                            start=(chi == 0),
                            stop=(chi == CHI - 1),
                        )
                    s2t = s2tp.tile([1, 4 * W], f32, tag="s2t")
                    nc.scalar.activation(
                        s2t[:, : ncr * W],
                        ss[:, : ncr * W],
                        mybir.ActivationFunctionType.Ln,
                    )
                    nc.scalar.activation(
                        s2g[0:1, s, s0 : s0 + ncr, 4 : 4 + W],
                        s2t[:, : ncr * W].rearrange("p (a b) -> p a b", b=W),
                        mybir.ActivationFunctionType.Exp,
                        scale=-0.5,
                    )
                    nc.gpsimd.partition_broadcast(
                        s2gB[:, s, s0 : s0 + ncr, :],
                        s2g[0:1, s, s0 : s0 + ncr, :],
                    )

            def zero_slots(s, slots):
                s0, n = slots[0], len(slots)
                zv2 = zeros.rearrange("p (a b) -> p a b", b=RW)
                for chi in range(CHI):
                    nc.sync.dma_start(ring[:, s, chi, s0 : s0 + n, :], zv2[:, 0:n, :])

            for b in range(NB):
                y0 = RB * b
                isb = isbe if b % 2 == 0 else isbo
                for s in range(SPC):
                    # --- f2 ring maintenance: ensure rows [y0-4, y0+12) ---
                    if b == 0:
                        zero_slots(s, [12, 13, 14, 15])  # rows -4..-1
                        load_f2_rows(s, list(range(0, 12)))
                    elif b == NB - 1:
                        load_f2_rows(s, list(range(y0 + 4, H)))
                        zero_slots(s, [0, 1, 2, 3])      # rows 128..131
                    else:
                        load_f2_rows(s, list(range(y0 + 4, y0 + 12)))

                    # --- f1 band (contiguous rows) + norms ---
                    f1t = f1p.tile([128, CHI, RB, W], f32r, tag="f1")
                    for chi in range(CHI):
                        nc.sync.dma_start(
                            f1t[:, chi], f1v[s, :, chi, y0 : y0 + RB, :]
                        )
                    # patch-major views/tiles: matmul stationary APs need ONE
                    # free dim, so re-layout on ACT (cheap big-op copies,
                    # split per chi: ACT ISA patterns allow max 3 free dims)
                    f1pm = f1pmp.tile([128, CHI, NP, RB, PW], bf16, tag="f1pm")
                    sq1 = sq1p.tile(
                        [128, CHI, NP, RB, PW], mybir.dt.float16, tag="sq1"
                    )
                    for chi in range(CHI):
                        f1tv = f1t[:, chi].bitcast(f32).rearrange(
                            "c r (np cx) -> c np r cx", cx=PW
                        )
                        nc.scalar.copy(f1pm[:, chi], f1tv)
                        nc.scalar.activation(
                            sq1[:, chi], f1tv, mybir.ActivationFunctionType.Square
                        )
                    s1ss = s1ps.tile([128, NP], f32, tag="s1ss")
                    for p in range(NP):
                        for chi in range(CHI):
                            nc.tensor.matmul(
                                s1ss[:, p : p + 1],
                                sq1[:, chi, p].rearrange("c a b -> c (a b)"),
                                onesb[:],
                                start=(chi == 0),
                                stop=(chi == CHI - 1),
                            )
                    s1l = s1p.tile([128, NP], f32, tag="s1l")
                    nc.scalar.activation(
                        s1l[:], s1ss[:], mybir.ActivationFunctionType.Ln
                    )
                    s1r = s1p.tile([128, NP], f32, tag="s1r")
                    nc.scalar.activation(
                        s1r[:], s1l[:], mybir.ActivationFunctionType.Exp, scale=-0.5
                    )

                    # --- output band tiles (bf16) ---
                    ht0 = htp.tile([108, RB, NP, PW], bf16, tag="ht0")
                    ht1 = htp.tile([108, RB, NP, PW], bf16, tag="ht1")

                    for p in range(NP):
                        x0 = PW * p
                        # Gram (bf16: lhsT from re-layout, rhs hi-half view)
                        g = gps.tile([128, SLOTS, WIN], f32, tag="g")
                        for chi in range(CHI):
                            rw = ring[:, s, chi, :, x0 : x0 + WIN].bitcast(
                                bf16
                            ).rearrange("c sl (x two) -> c sl x two", two=2)[
                                :, :, :, 1
                            ]
                            nc.tensor.matmul(
                                g[:].rearrange("p s x -> p (s x)"),
                                f1pm[:, chi, p].rearrange("c a b -> c (a b)"),
                                rw,
                                start=(chi == 0),
                                stop=(chi == CHI - 1),
                            )
                        # fused normalization
                        data = datap.tile([128, GN], f32, tag="data")
                        nc.vector.scalar_tensor_tensor(
                            data[:].rearrange("p (a b) -> p a b", b=WIN),
                            g[:],
                            s1r[:, p : p + 1],
                            s2gB[:, s, :, x0 : x0 + WIN],
                            op0=mybir.AluOpType.mult,
                            op1=mybir.AluOpType.mult,
                        )
                        # realign window rows per pixel-row (shared idx per r)
                        ht = hp.tile([128, HFREE], f32, tag="h")
                        nc.gpsimd.indirect_copy(
                            ht[:].rearrange("p (a b) -> p a b", b=INNER),
                            data[:].rearrange("p (a b) -> p a b", b=INNER),
                            isb[:],
                            True,
                        )
                        # transpose the 216 used planes via bf16 view (1 cyc/row)
                        htv = hi_bf16(ht[:])
                        tp = tps.tile([108, 256], bf16, tag="t")
                        nc.tensor.transpose(tp[:, 0:128], htv[:, 0:108], idsb[:])
                        nc.tensor.transpose(tp[:, 128:256], htv[:, 108:216], idsb[:])
                        nc.scalar.copy(ht0[:, :, p, :], tp[:, 0:128].rearrange(
                            "d (r cx) -> d r cx", cx=PW))
                        nc.vector.tensor_copy(ht1[:, :, p, :], tp[:, 128:256].rearrange(
                            "d (r cx) -> d r cx", cx=PW))

                    opv = opad[s, :, y0 : y0 + RB, :].rearrange(
                        "d r (p cx) -> d r p cx", cx=PW
                    )
                    nc.scalar.dma_start(opv[0:108], ht0[:])
                    nc.scalar.dma_start(opv[108:216], ht1[:])

    nc.compile()
    return nc


def _constants():
    import ml_dtypes

    return {
        "idx_e": _idx_table(0),
        "idx_o": _idx_table(1),
        "ident": np.eye(128, dtype=ml_dtypes.bfloat16),
        "ones": np.ones((128, 1), dtype=np.float16),
        "zeros": np.zeros((128, 1088), dtype=np.float32),
    }


def _unscramble(pad: np.ndarray) -> np.ndarray:
    """pad [SPC, 216, H, W] -> out [SPC, 81, H, W].

    pad[s, a*24+e, y, x] = corr for dy = 4-a, dx = 4-(e - x%16)."""
    pad = np.asarray(pad).astype(np.float32)
    pad = pad.reshape(SPC, NA, WIN, H, W)
    tmp = np.empty((SPC, NA, NA, H, W), dtype=pad.dtype)
    for xm in range(PW):
        tmp[:, :, :, :, xm::PW] = pad[:, :, xm : xm + NA, :, xm::PW]
    # d order: (dy+4, dx+4) = (8-a, 8-adx)
    return tmp[:, ::-1, ::-1].reshape(SPC, NA * NA, H, W)


def kernel(feat1: np.ndarray, feat2: np.ndarray) -> np.ndarray:
    from concourse.bass_utils import run_bass_kernel_spmd

    feat1 = np.ascontiguousarray(np.asarray(feat1, dtype=np.float32))
    feat2 = np.ascontiguousarray(np.asarray(feat2, dtype=np.float32))

    if "nc" not in _CACHE:
        _CACHE["nc"] = _build_nc()
    nc = _CACHE["nc"]

    consts = _constants()
    in_maps = []
    for core in range(NCORES):
        s0 = SPC * core
        m = {"feat1": feat1[s0 : s0 + SPC], "feat2": feat2[s0 : s0 + SPC]}
        m.update(consts)
        in_maps.append(m)

    trace = _CACHE.get("trace", False)
    res = run_bass_kernel_spmd(nc, in_maps, list(range(NCORES)), trace=trace)
    _CACHE["last_results"] = res

    out = np.empty((B, NA * NA, H, W), dtype=np.float32)
    for core in range(NCORES):
        s0 = SPC * core
        out[s0 : s0 + SPC] = _unscramble(res.results[core]["out_pad"])
    return out


# revision 10
# speedup vs baseline: 1.3519x; 1.1704x over previous
"""Trainium2 Bass kernel for nn_CorrelationLayer: B=16, C=256, H=W=128, max_disp=4.

out[b, (dy+4)*9+(dx+4), y, x] = sum_c f1n[b,c,y,x] * f2n[b,c,y-dy,x-dx]
with f1n/f2n L2-normalized over c (zero-fill outside the image).

Strategy (per core, 2 samples, pure batch data-parallel over 8 cores):
  Bands (8 rows) x patches (8x16 px). Per patch the PE computes a banded
  Gram G[128 px, 16 slots * 24 wx] against a 16-row f2 ring buffer
  (K=256 via 2-pass PSUM accumulation, f32r operands).
  DVE fuses normalization: data = G * rsqrt(|f1[px]|^2) * rsqrt(|f2[win]|^2)
  (row scalar per partition + column vector broadcast). rsqrt is computed
  on ACT as exp(-0.5*ln(x)) to avoid the 1-lane DVE reciprocal.
  GPSIMD indirect_copy realigns window rows per pixel-row -> H[128, 9*24].
  PE transposes H via a truncated-bf16 bitcast view (1 cyc/row) ->
  [216 planes, 128 px]; ACT+DVE copy halves into bf16 band tiles; DMA
  stores to a padded bf16 output [216, H, W] (2KB runs). Host converts to
  f32 and unscrambles e -> dx, reversing both disp axes.

  The two samples of a core are processed interleaved (independent rings)
  so each sample's f2 ring refill + norm chain overlaps the other
  sample's compute instead of stalling on ring reuse.
"""

import sys

sys.path.insert(0, "/opt/trn_rl_repo")

import numpy as np

B, C, H, W = 16, 256, 128, 128
NCORES = 8
SPC = B // NCORES        # samples per core
CHI = 2                  # c = chi*128 + partition
NB = 16                  # bands per image
RB = 8                   # rows per band
NP = 8                   # patches per band
PW = 16                  # patch width
WIN = 24                 # f2 window width per patch (16 + 8)
SLOTS = 16               # ring rows
RW = 136                 # ring width (128 + 2*4 zero pad)
NA = 9                   # row displacements (a = 4 - dy)
NEB = 3                  # 8-element blocks per window row
INNER = 8
NIDX = 28                # 27 used + 1 pad (alignment)
HFREE = NIDX * INNER     # 224
USED = NA * WIN          # 216
GN = SLOTS * WIN         # gram free size 384

_CACHE = {}


def _idx_table(parity: int) -> np.ndarray:
    t = np.zeros((128, 2), dtype=np.uint16)
    for r in range(RB):
        for j in range(NA * NEB):
            a, eb = divmod(j, NEB)
            slot = (8 * parity + r + a + 12) % SLOTS
            t[16 * r + (j % 16), j // 16] = slot * WIN + INNER * eb
    return t


def _build_nc():
    import concourse.tile as tile
    from concourse import bacc, mybir

    f32 = mybir.dt.float32
    f32r = mybir.dt.float32r
    bf16 = mybir.dt.bfloat16

    nc = bacc.Bacc("TRN2", target_bir_lowering=False, debug=False)

    f1d = nc.dram_tensor("feat1", [SPC, C, H, W], f32r, kind="ExternalInput").ap()
    f2d = nc.dram_tensor("feat2", [SPC, C, H, W], f32r, kind="ExternalInput").ap()
    idx_e = nc.dram_tensor("idx_e", [128, 2], mybir.dt.uint16, kind="ExternalInput").ap()
    idx_o = nc.dram_tensor("idx_o", [128, 2], mybir.dt.uint16, kind="ExternalInput").ap()
    ident = nc.dram_tensor("ident", [128, 128], bf16, kind="ExternalInput").ap()
    ones = nc.dram_tensor("ones", [128, 1], mybir.dt.float16, kind="ExternalInput").ap()
    zeros = nc.dram_tensor("zeros", [128, 1088], f32r, kind="ExternalInput").ap()
    opad = nc.dram_tensor("out_pad", [SPC, USED, H, W], bf16, kind="ExternalOutput").ap()

    f1v = f1d.rearrange("s (chi p) y x -> s p chi y x", p=128)
    f2v = f2d.rearrange("s (chi p) y x -> s p chi y x", p=128)

    def act_rsqrt(out, in_):
        eng = nc.scalar
        bias = nc.const_aps.scalar_like(0.0, in_)
        return eng.add_instruction(
            mybir.InstActivation(
                name=nc.get_next_instruction_name(),
                func=mybir.ActivationFunctionType.Rsqrt,
                ins=[
                    eng.lower_ap(in_),
                    eng.lower_ap(bias),
                    mybir.ImmediateValue(dtype=f32, value=1.0),
                    mybir.ImmediateValue(dtype=f32, value=0.0),
                ],
                outs=[eng.lower_ap(out)],
            )
        )

    def hi_bf16(ap):
        """Truncated-bf16 view of an f32 AP (high half of each word)."""
        n = ap.shape[-1]
        return ap.bitcast(bf16).rearrange(
            "p (n two) -> p n two", two=2
        )[:, :, 1]

    with tile.TileContext(nc) as tc:
        with (
            tc.tile_pool(name="persist", bufs=1) as pers,
            tc.tile_pool(name="f1p", bufs=2) as f1p,
            tc.tile_pool(name="f1pmp", bufs=2) as f1pmp,
            tc.tile_pool(name="sq1p", bufs=2) as sq1p,
            tc.tile_pool(name="sq2p", bufs=2) as sq2p,
            tc.tile_pool(name="s1p", bufs=2) as s1p,
            tc.tile_pool(name="datap", bufs=3) as datap,
            tc.tile_pool(name="hp", bufs=3) as hp,
            tc.tile_pool(name="htp", bufs=2) as htp,
            tc.tile_pool(name="gps", bufs=2, space="PSUM") as gps,
            tc.tile_pool(name="tpps", bufs=1, space="PSUM") as tpps,
            tc.tile_pool(name="s1ps", bufs=2, space="PSUM") as s1ps,
            tc.tile_pool(name="s2ps", bufs=2, space="PSUM") as s2ps,
        ):
            # persistent tiles (per-sample rings)
            ring = pers.tile([128, SPC, CHI, SLOTS, RW], f32r)   # f2 rows
            s2g = pers.tile([1, SPC, SLOTS, RW], f32)            # rsqrt(|f2 row|^2)
            s2gB = pers.tile([128, SPC, SLOTS, RW], f32)         # bcast over partitions
            isbe = pers.tile([128, 2], mybir.dt.uint16)
            isbo = pers.tile([128, 2], mybir.dt.uint16)
            idsb = pers.tile([128, 128], bf16)
            onesb = pers.tile([128, 1], mybir.dt.float16)

            nc.sync.dma_start(isbe[:], idx_e)
            nc.sync.dma_start(isbo[:], idx_o)
            nc.sync.dma_start(idsb[:], ident)
            nc.sync.dma_start(onesb[:], ones)

            # zero x-pad columns of the rings (once; loads never touch them)
            zv = zeros.rearrange("p (a b) -> p a b", b=4)
            for s in range(SPC):
                for chi in range(CHI):
                    nc.sync.dma_start(ring[:, s, chi, :, 0:4], zv[:, 0:SLOTS, :])
                    nc.sync.dma_start(ring[:, s, chi, :, 132:136], zv[:, 0:SLOTS, :])
            # 1.0 (not 0.0): s2gB is a divisor in the STT; pads/unloaded
            # slots must divide G=0 by a nonzero value (0/0 would NaN)
            nc.vector.memset(s2g[:], 1.0)
            nc.vector.memset(s2gB[:], 1.0)

            def load_f2_rows(s, rows):
                """DMA f2 rows (consecutive image rows) into ring slots, plus
                squares -> norms -> rsqrt -> broadcast. rows are 4-aligned."""
                i = 0
                while i < len(rows):
                    j = i
                    while (
                        j + 1 < len(rows)
                        and rows[j + 1] % SLOTS == (rows[j] % SLOTS) + 1
                    ):
                        j += 1
                    run = rows[i : j + 1]
                    s0 = run[0] % SLOTS
                    n = len(run)
                    for chi in range(CHI):
                        nc.sync.dma_start(
                            ring[:, s, chi, s0 : s0 + n, 4 : 4 + W],
                            f2v[s, :, chi, run[0] : run[0] + n, :],
                        )
                    i = j + 1
                # norms in 4-row chunks: rsqrt(ss) = exp(-0.5 * ln(ss))
                for i in range(0, len(rows), 4):
                    chunk = rows[i : i + 4]
                    s0 = chunk[0] % SLOTS
                    ncr = len(chunk)
                    sq2 = sq2p.tile([128, CHI, 4, W], mybir.dt.float16, tag="sq2")
                    nc.scalar.activation(
                        sq2[:, :, :ncr, :],
                        ring[:, s, :, s0 : s0 + ncr, 4 : 4 + W].bitcast(f32),
                        mybir.ActivationFunctionType.Square,
                    )
                    ss = s2ps.tile([1, 4 * W], f32, tag="s2ss")
                    for chi in range(CHI):
                        nc.tensor.matmul(
                            ss[:, : ncr * W],
                            onesb[:],
                            sq2[:, chi, :ncr, :],
                            start=(chi == 0),
                            stop=(chi == CHI - 1),
                        )
                    act_rsqrt(
                        s2g[0:1, s, s0 : s0 + ncr, 4 : 4 + W],
                        ss[:, : ncr * W].rearrange("p (a b) -> p a b", b=W),
                    )
                    nc.gpsimd.partition_broadcast(
                        s2gB[:, s, s0 : s0 + ncr, :],
                        s2g[0:1, s, s0 : s0 + ncr, :],
                    )

            def zero_slots(s, slots):
                s0, n = slots[0], len(slots)
                zv2 = zeros.rearrange("p (a b) -> p a b", b=RW)
                for chi in range(CHI):
                    nc.sync.dma_start(ring[:, s, chi, s0 : s0 + n, :], zv2[:, 0:n, :])

            for b in range(NB):
                y0 = RB * b
                isb = isbe if b % 2 == 0 else isbo
                for s in range(SPC):
                    # --- f2 ring maintenance: ensure rows [y0-4, y0+12) ---
                    if b == 0:
                        zero_slots(s, [12, 13, 14, 15])  # rows -4..-1
                        load_f2_rows(s, list(range(0, 12)))
                    elif b == NB - 1:
                        load_f2_rows(s, list(range(y0 + 4, H)))
                        zero_slots(s, [0, 1, 2, 3])      # rows 128..131
                    else:
                        load_f2_rows(s, list(range(y0 + 4, y0 + 12)))

                    # --- f1 band (contiguous rows) + norms ---
                    f1t = f1p.tile([128, CHI, RB, W], f32r, tag="f1")
                    for chi in range(CHI):
                        nc.sync.dma_start(
                            f1t[:, chi], f1v[s, :, chi, y0 : y0 + RB, :]
                        )
                    # patch-major views/tiles: matmul stationary APs need ONE
                    # free dim, so re-layout on ACT (cheap big-op copies,
                    # split per chi: ACT ISA patterns allow max 3 free dims)
                    f1pm = f1pmp.tile([128, CHI, NP, RB, PW], bf16, tag="f1pm")
                    sq1 = sq1p.tile(
                        [128, CHI, NP, RB, PW], mybir.dt.float16, tag="sq1"
                    )
                    for chi in range(CHI):
                        f1tv = f1t[:, chi].bitcast(f32).rearrange(
                            "c r (np cx) -> c np r cx", cx=PW
                        )
                        nc.scalar.copy(f1pm[:, chi], f1tv)
                        nc.scalar.activation(
                            sq1[:, chi], f1tv, mybir.ActivationFunctionType.Square
                        )
                    s1ss = s1ps.tile([128, NP], f32, tag="s1ss")
                    for p in range(NP):
                        for chi in range(CHI):
                            nc.tensor.matmul(
                                s1ss[:, p : p + 1],
                                sq1[:, chi, p].rearrange("c a b -> c (a b)"),
                                onesb[:],
                                start=(chi == 0),
                                stop=(chi == CHI - 1),
                            )
                    s1r = s1p.tile([128, NP], f32, tag="s1r")
                    act_rsqrt(s1r[:], s1ss[:])

                    # --- output band tiles (bf16) ---
                    ht0 = htp.tile([108, RB, NP, PW], bf16, tag="ht0")
                    ht1 = htp.tile([108, RB, NP, PW], bf16, tag="ht1")
                    tpb = tpps.tile([108, NP, 2, 128], bf16, tag="tpb")

                    for p in range(NP):
                        x0 = PW * p
                        # Gram (bf16: lhsT from re-layout, rhs hi-half view)
                        g = gps.tile([128, SLOTS, WIN], f32, tag="g")
                        for chi in range(CHI):
                            rw = ring[:, s, chi, :, x0 : x0 + WIN].bitcast(
                                bf16
                            ).rearrange("c sl (x two) -> c sl x two", two=2)[
                                :, :, :, 1
                            ]
                            nc.tensor.matmul(
                                g[:].rearrange("p s x -> p (s x)"),
                                f1pm[:, chi, p].rearrange("c a b -> c (a b)"),
                                rw,
                                start=(chi == 0),
                                stop=(chi == CHI - 1),
                            )
                        # fused normalization
                        data = datap.tile([128, GN], f32, tag="data")
                        nc.vector.scalar_tensor_tensor(
                            data[:].rearrange("p (a b) -> p a b", b=WIN),
                            g[:],
                            s1r[:, p : p + 1],
                            s2gB[:, s, :, x0 : x0 + WIN],
                            op0=mybir.AluOpType.mult,
                            op1=mybir.AluOpType.mult,
                        )
                        # realign window rows per pixel-row (shared idx per r)
                        ht = hp.tile([128, HFREE], f32, tag="h")
                        nc.gpsimd.indirect_copy(
                            ht[:].rearrange("p (a b) -> p a b", b=INNER),
                            data[:].rearrange("p (a b) -> p a b", b=INNER),
                            isb[:],
                            True,
                        )
                        # transpose the 216 used planes via bf16 view (1 cyc/row)
                        htv = hi_bf16(ht[:])
                        nc.tensor.transpose(tpb[:, p, 0], htv[:, 0:108], idsb[:])
                        nc.tensor.transpose(tpb[:, p, 1], htv[:, 108:216], idsb[:])

                    # batched PSUM drain: one big copy per half (ACT + DVE)
                    nc.scalar.copy(ht0[:], tpb[:, :, 0].rearrange(
                        "d p (r cx) -> d r p cx", cx=PW))
                    nc.vector.tensor_copy(ht1[:], tpb[:, :, 1].rearrange(
                        "d p (r cx) -> d r p cx", cx=PW))

                    opv = opad[s, :, y0 : y0 + RB, :].rearrange(
                        "d r (p cx) -> d r p cx", cx=PW
                    )
                    nc.scalar.dma_start(opv[0:108], ht0[:])
                    nc.scalar.dma_start(opv[108:216], ht1[:])

    nc.compile()
    return nc


def _constants():
    import ml_dtypes

    return {
        "idx_e": _idx_table(0),
        "idx_o": _idx_table(1),
        "ident": np.eye(128, dtype=ml_dtypes.bfloat16),
        "ones": np.ones((128, 1), dtype=np.float16),
        "zeros": np.zeros((128, 1088), dtype=np.float32),
    }


def _unscramble(pad: np.ndarray) -> np.ndarray:
    """pad [SPC, 216, H, W] -> out [SPC, 81, H, W].

    pad[s, a*24+e, y, x] = corr for dy = 4-a, dx = 4-(e - x%16)."""
    pad = np.asarray(pad).astype(np.float32)
    pad = pad.reshape(SPC, NA, WIN, H, W)
    tmp = np.empty((SPC, NA, NA, H, W), dtype=pad.dtype)
    for xm in range(PW):
        tmp[:, :, :, :, xm::PW] = pad[:, :, xm : xm + NA, :, xm::PW]
    # d order: (dy+4, dx+4) = (8-a, 8-adx)
    return tmp[:, ::-1, ::-1].reshape(SPC, NA * NA, H, W)


def kernel(feat1: np.ndarray, feat2: np.ndarray) -> np.ndarray:
    from concourse.bass_utils import run_bass_kernel_spmd

    feat1 = np.ascontiguousarray(np.asarray(feat1, dtype=np.float32))
    feat2 = np.ascontiguousarray(np.asarray(feat2, dtype=np.float32))

    if "nc" not in _CACHE:
        _CACHE["nc"] = _build_nc()
    nc = _CACHE["nc"]

    consts = _constants()
    in_maps = []
    for core in range(NCORES):
        s0 = SPC * core
        m = {"feat1": feat1[s0 : s0 + SPC], "feat2": feat2[s0 : s0 + SPC]}
        m.update(consts)
        in_maps.append(m)

    trace = _CACHE.get("trace", False)
    res = run_bass_kernel_spmd(nc, in_maps, list(range(NCORES)), trace=trace)
    _CACHE["last_results"] = res

    out = np.empty((B, NA * NA, H, W), dtype=np.float32)
    for core in range(NCORES):
        s0 = SPC * core
        out[s0 : s0 + SPC] = _unscramble(res.results[core]["out_pad"])
    return out


# revision 11
# speedup vs baseline: 1.3737x; 1.0162x over previous
"""Trainium2 Bass kernel for nn_CorrelationLayer: B=16, C=256, H=W=128, max_disp=4.

out[b, (dy+4)*9+(dx+4), y, x] = sum_c f1n[b,c,y,x] * f2n[b,c,y-dy,x-dx]
with f1n/f2n L2-normalized over c (zero-fill outside the image).

Strategy (per core, 2 samples, pure batch data-parallel over 8 cores):
  Bands (8 rows) x patches (8x16 px). Per patch the PE computes a banded
  Gram G[128 px, 16 slots * 24 wx] against a 16-row f2 ring buffer
  (K=256 via 2-pass PSUM accumulation, f32r operands).
  DVE fuses normalization: data = G * rsqrt(|f1[px]|^2) * rsqrt(|f2[win]|^2)
  (row scalar per partition + column vector broadcast). rsqrt is computed
  on ACT as exp(-0.5*ln(x)) to avoid the 1-lane DVE reciprocal.
  GPSIMD indirect_copy realigns window rows per pixel-row -> H[128, 9*24].
  PE transposes H via a truncated-bf16 bitcast view (1 cyc/row) ->
  [216 planes, 128 px]; ACT+DVE copy halves into bf16 band tiles; DMA
  stores to a padded bf16 output [216, H, W] (2KB runs). Host converts to
  f32 and unscrambles e -> dx, reversing both disp axes.

  The two samples of a core are processed interleaved (independent rings)
  so each sample's f2 ring refill + norm chain overlaps the other
  sample's compute instead of stalling on ring reuse.
"""

import sys

sys.path.insert(0, "/opt/trn_rl_repo")

import numpy as np

B, C, H, W = 16, 256, 128, 128
NCORES = 8
SPC = B // NCORES        # samples per core
CHI = 2                  # c = chi*128 + partition
NB = 16                  # bands per image
RB = 8                   # rows per band
NP = 8                   # patches per band
PW = 16                  # patch width
WIN = 24                 # f2 window width per patch (16 + 8)
SLOTS = 16               # ring rows
RW = 136                 # ring width (128 + 2*4 zero pad)
NA = 9                   # row displacements (a = 4 - dy)
NEB = 3                  # 8-element blocks per window row
INNER = 8
NIDX = 28                # 27 used + 1 pad (alignment)
HFREE = NIDX * INNER     # 224
USED = NA * WIN          # 216
GN = SLOTS * WIN         # gram free size 384

_CACHE = {}


def _idx_table(parity: int) -> np.ndarray:
    t = np.zeros((128, 2), dtype=np.uint16)
    for r in range(RB):
        for j in range(NA * NEB):
            a, eb = divmod(j, NEB)
            slot = (8 * parity + r + a + 12) % SLOTS
            t[16 * r + (j % 16), j // 16] = slot * WIN + INNER * eb
    return t


def _build_nc():
    import concourse.tile as tile
    from concourse import bacc, mybir

    f32 = mybir.dt.float32
    f32r = mybir.dt.float32r
    bf16 = mybir.dt.bfloat16

    nc = bacc.Bacc("TRN2", target_bir_lowering=False, debug=False)

    f1d = nc.dram_tensor("feat1", [SPC, C, H, W], f32r, kind="ExternalInput").ap()
    f2d = nc.dram_tensor("feat2", [SPC, C, H, W], f32r, kind="ExternalInput").ap()
    idx_e = nc.dram_tensor("idx_e", [128, 2], mybir.dt.uint16, kind="ExternalInput").ap()
    idx_o = nc.dram_tensor("idx_o", [128, 2], mybir.dt.uint16, kind="ExternalInput").ap()
    ident = nc.dram_tensor("ident", [128, 128], bf16, kind="ExternalInput").ap()
    ones = nc.dram_tensor("ones", [128, 1], mybir.dt.float16, kind="ExternalInput").ap()
    zeros = nc.dram_tensor("zeros", [128, 1088], f32r, kind="ExternalInput").ap()
    opad = nc.dram_tensor("out_pad", [SPC, USED, H, W], bf16, kind="ExternalOutput").ap()

    f1v = f1d.rearrange("s (chi p) y x -> s p chi y x", p=128)
    f2v = f2d.rearrange("s (chi p) y x -> s p chi y x", p=128)

    def act_rsqrt(out, in_):
        eng = nc.scalar
        bias = nc.const_aps.scalar_like(0.0, in_)
        return eng.add_instruction(
            mybir.InstActivation(
                name=nc.get_next_instruction_name(),
                func=mybir.ActivationFunctionType.Rsqrt,
                ins=[
                    eng.lower_ap(in_),
                    eng.lower_ap(bias),
                    mybir.ImmediateValue(dtype=f32, value=1.0),
                    mybir.ImmediateValue(dtype=f32, value=0.0),
                ],
                outs=[eng.lower_ap(out)],
            )
        )

    def hi_bf16(ap):
        """Truncated-bf16 view of an f32 AP (high half of each word)."""
        n = ap.shape[-1]
        return ap.bitcast(bf16).rearrange(
            "p (n two) -> p n two", two=2
        )[:, :, 1]

    with tile.TileContext(nc) as tc:
        with (
            tc.tile_pool(name="persist", bufs=1) as pers,
            tc.tile_pool(name="f1p", bufs=2) as f1p,
            tc.tile_pool(name="f1pmp", bufs=2) as f1pmp,
            tc.tile_pool(name="sq1p", bufs=2) as sq1p,
            tc.tile_pool(name="sq2p", bufs=2) as sq2p,
            tc.tile_pool(name="s1p", bufs=2) as s1p,
            tc.tile_pool(name="datap", bufs=3) as datap,
            tc.tile_pool(name="hp", bufs=3) as hp,
            tc.tile_pool(name="htp", bufs=2) as htp,
            tc.tile_pool(name="gps", bufs=2, space="PSUM") as gps,
            tc.tile_pool(name="tpps", bufs=2, space="PSUM") as tpps,
            tc.tile_pool(name="s1ps", bufs=1, space="PSUM") as s1ps,
            tc.tile_pool(name="s2ps", bufs=1, space="PSUM") as s2ps,
        ):
            # persistent tiles (per-sample rings)
            ring = pers.tile([128, SPC, CHI, SLOTS, RW], f32r)   # f2 rows
            s2g = pers.tile([1, SPC, SLOTS, RW], f32)            # rsqrt(|f2 row|^2)
            s2gB = pers.tile([128, SPC, SLOTS, RW], f32)         # bcast over partitions
            isbe = pers.tile([128, 2], mybir.dt.uint16)
            isbo = pers.tile([128, 2], mybir.dt.uint16)
            idsb = pers.tile([128, 128], bf16)
            onesb = pers.tile([128, 1], mybir.dt.float16)

            nc.sync.dma_start(isbe[:], idx_e)
            nc.sync.dma_start(isbo[:], idx_o)
            nc.sync.dma_start(idsb[:], ident)
            nc.sync.dma_start(onesb[:], ones)

            # zero x-pad columns of the rings (once; loads never touch them)
            zv = zeros.rearrange("p (a b) -> p a b", b=4)
            for s in range(SPC):
                for chi in range(CHI):
                    nc.sync.dma_start(ring[:, s, chi, :, 0:4], zv[:, 0:SLOTS, :])
                    nc.sync.dma_start(ring[:, s, chi, :, 132:136], zv[:, 0:SLOTS, :])
            # 1.0 (not 0.0): s2gB is a divisor in the STT; pads/unloaded
            # slots must divide G=0 by a nonzero value (0/0 would NaN)
            nc.vector.memset(s2g[:], 1.0)
            nc.vector.memset(s2gB[:], 1.0)

            def load_f2_rows(s, rows):
                """DMA f2 rows (consecutive image rows) into ring slots, plus
                squares -> norms -> rsqrt -> broadcast. rows are 4-aligned."""
                i = 0
                while i < len(rows):
                    j = i
                    while (
                        j + 1 < len(rows)
                        and rows[j + 1] % SLOTS == (rows[j] % SLOTS) + 1
                    ):
                        j += 1
                    run = rows[i : j + 1]
                    s0 = run[0] % SLOTS
                    n = len(run)
                    for chi in range(CHI):
                        nc.sync.dma_start(
                            ring[:, s, chi, s0 : s0 + n, 4 : 4 + W],
                            f2v[s, :, chi, run[0] : run[0] + n, :],
                        )
                    i = j + 1
                # norms in 4-row chunks: rsqrt(ss) = exp(-0.5 * ln(ss))
                for i in range(0, len(rows), 4):
                    chunk = rows[i : i + 4]
                    s0 = chunk[0] % SLOTS
                    ncr = len(chunk)
                    sq2 = sq2p.tile([128, CHI, 4, W], mybir.dt.float16, tag="sq2")
                    nc.scalar.activation(
                        sq2[:, :, :ncr, :],
                        ring[:, s, :, s0 : s0 + ncr, 4 : 4 + W].bitcast(f32),
                        mybir.ActivationFunctionType.Square,
                    )
                    ss = s2ps.tile([1, 4 * W], f32, tag="s2ss")
                    for chi in range(CHI):
                        nc.tensor.matmul(
                            ss[:, : ncr * W],
                            onesb[:],
                            sq2[:, chi, :ncr, :],
                            start=(chi == 0),
                            stop=(chi == CHI - 1),
                        )
                    act_rsqrt(
                        s2g[0:1, s, s0 : s0 + ncr, 4 : 4 + W],
                        ss[:, : ncr * W].rearrange("p (a b) -> p a b", b=W),
                    )
                    nc.gpsimd.partition_broadcast(
                        s2gB[:, s, s0 : s0 + ncr, :],
                        s2g[0:1, s, s0 : s0 + ncr, :],
                    )

            def zero_slots(s, slots):
                s0, n = slots[0], len(slots)
                zv2 = zeros.rearrange("p (a b) -> p a b", b=RW)
                for chi in range(CHI):
                    nc.sync.dma_start(ring[:, s, chi, s0 : s0 + n, :], zv2[:, 0:n, :])

            for b in range(NB):
                y0 = RB * b
                isb = isbe if b % 2 == 0 else isbo
                for s in range(SPC):
                    # --- f2 ring maintenance: ensure rows [y0-4, y0+12) ---
                    if b == 0:
                        zero_slots(s, [12, 13, 14, 15])  # rows -4..-1
                        load_f2_rows(s, list(range(0, 12)))
                    elif b == NB - 1:
                        load_f2_rows(s, list(range(y0 + 4, H)))
                        zero_slots(s, [0, 1, 2, 3])      # rows 128..131
                    else:
                        load_f2_rows(s, list(range(y0 + 4, y0 + 12)))

                    # --- f1 band (contiguous rows) + norms ---
                    f1t = f1p.tile([128, CHI, RB, W], f32r, tag="f1")
                    for chi in range(CHI):
                        nc.sync.dma_start(
                            f1t[:, chi], f1v[s, :, chi, y0 : y0 + RB, :]
                        )
                    # patch-major views/tiles: matmul stationary APs need ONE
                    # free dim, so re-layout on ACT (cheap big-op copies,
                    # split per chi: ACT ISA patterns allow max 3 free dims)
                    f1pm = f1pmp.tile([128, CHI, NP, RB, PW], bf16, tag="f1pm")
                    sq1 = sq1p.tile(
                        [128, CHI, NP, RB, PW], mybir.dt.float16, tag="sq1"
                    )
                    for chi in range(CHI):
                        f1tv = f1t[:, chi].bitcast(f32).rearrange(
                            "c r (np cx) -> c np r cx", cx=PW
                        )
                        nc.scalar.copy(f1pm[:, chi], f1tv)
                        nc.scalar.activation(
                            sq1[:, chi], f1tv, mybir.ActivationFunctionType.Square
                        )
                    s1ss = s1ps.tile([128, NP], f32, tag="s1ss")
                    for p in range(NP):
                        for chi in range(CHI):
                            nc.tensor.matmul(
                                s1ss[:, p : p + 1],
                                sq1[:, chi, p].rearrange("c a b -> c (a b)"),
                                onesb[:],
                                start=(chi == 0),
                                stop=(chi == CHI - 1),
                            )
                    s1r = s1p.tile([128, NP], f32, tag="s1r")
                    act_rsqrt(s1r[:], s1ss[:])

                    # --- output band tiles (bf16) ---
                    ht0 = htp.tile([108, RB, NP, PW], bf16, tag="ht0")
                    ht1 = htp.tile([108, RB, NP, PW], bf16, tag="ht1")
                    tpb = tpps.tile([108, NP, 2, 128], bf16, tag="tpb")

                    for p in range(NP):
                        x0 = PW * p
                        # Gram (bf16: lhsT from re-layout, rhs hi-half view)
                        g = gps.tile([128, SLOTS, WIN], f32, tag="g")
                        for chi in range(CHI):
                            rw = ring[:, s, chi, :, x0 : x0 + WIN].bitcast(
                                bf16
                            ).rearrange("c sl (x two) -> c sl x two", two=2)[
                                :, :, :, 1
                            ]
                            nc.tensor.matmul(
                                g[:].rearrange("p s x -> p (s x)"),
                                f1pm[:, chi, p].rearrange("c a b -> c (a b)"),
                                rw,
                                start=(chi == 0),
                                stop=(chi == CHI - 1),
                            )
                        # fused normalization
                        data = datap.tile([128, GN], f32, tag="data")
                        nc.vector.scalar_tensor_tensor(
                            data[:].rearrange("p (a b) -> p a b", b=WIN),
                            g[:],
                            s1r[:, p : p + 1],
                            s2gB[:, s, :, x0 : x0 + WIN],
                            op0=mybir.AluOpType.mult,
                            op1=mybir.AluOpType.mult,
                        )
                        # realign window rows per pixel-row (shared idx per r)
                        ht = hp.tile([128, HFREE], f32, tag="h")
                        nc.gpsimd.indirect_copy(
                            ht[:].rearrange("p (a b) -> p a b", b=INNER),
                            data[:].rearrange("p (a b) -> p a b", b=INNER),
                            isb[:],
                            True,
                        )
                        # transpose the 216 used planes via bf16 view (1 cyc/row)
                        htv = hi_bf16(ht[:])
                        nc.tensor.transpose(tpb[:, p, 0], htv[:, 0:108], idsb[:])
                        nc.tensor.transpose(tpb[:, p, 1], htv[:, 108:216], idsb[:])

                    # batched PSUM drain: one big copy per half (ACT + DVE)
                    nc.scalar.copy(ht0[:], tpb[:, :, 0].rearrange(
                        "d p (r cx) -> d r p cx", cx=PW))
                    nc.vector.tensor_copy(ht1[:], tpb[:, :, 1].rearrange(
                        "d p (r cx) -> d r p cx", cx=PW))

                    opv = opad[s, :, y0 : y0 + RB, :].rearrange(
                        "d r (p cx) -> d r p cx", cx=PW
                    )
                    nc.scalar.dma_start(opv[0:108], ht0[:])
                    nc.scalar.dma_start(opv[108:216], ht1[:])

    nc.compile()
    return nc


def _constants():
    import ml_dtypes

    return {
        "idx_e": _idx_table(0),
        "idx_o": _idx_table(1),
        "ident": np.eye(128, dtype=ml_dtypes.bfloat16),
        "ones": np.ones((128, 1), dtype=np.float16),
        "zeros": np.zeros((128, 1088), dtype=np.float32),
    }


def _unscramble(pad: np.ndarray) -> np.ndarray:
    """pad [SPC, 216, H, W] -> out [SPC, 81, H, W].

    pad[s, a*24+e, y, x] = corr for dy = 4-a, dx = 4-(e - x%16)."""
    pad = np.asarray(pad).astype(np.float32)
    pad = pad.reshape(SPC, NA, WIN, H, W)
    tmp = np.empty((SPC, NA, NA, H, W), dtype=pad.dtype)
    for xm in range(PW):
        tmp[:, :, :, :, xm::PW] = pad[:, :, xm : xm + NA, :, xm::PW]
    # d order: (dy+4, dx+4) = (8-a, 8-adx)
    return tmp[:, ::-1, ::-1].reshape(SPC, NA * NA, H, W)


def kernel(feat1: np.ndarray, feat2: np.ndarray) -> np.ndarray:
    from concourse.bass_utils import run_bass_kernel_spmd

    feat1 = np.ascontiguousarray(np.asarray(feat1, dtype=np.float32))
    feat2 = np.ascontiguousarray(np.asarray(feat2, dtype=np.float32))

    if "nc" not in _CACHE:
        _CACHE["nc"] = _build_nc()
    nc = _CACHE["nc"]

    consts = _constants()
    in_maps = []
    for core in range(NCORES):
        s0 = SPC * core
        m = {"feat1": feat1[s0 : s0 + SPC], "feat2": feat2[s0 : s0 + SPC]}
        m.update(consts)
        in_maps.append(m)

    trace = _CACHE.get("trace", False)
    res = run_bass_kernel_spmd(nc, in_maps, list(range(NCORES)), trace=trace)
    _CACHE["last_results"] = res

    out = np.empty((B, NA * NA, H, W), dtype=np.float32)
    for core in range(NCORES):
        s0 = SPC * core
        out[s0 : s0 + SPC] = _unscramble(res.results[core]["out_pad"])
    return out


# revision 13
# speedup vs baseline: 1.3791x; 1.0039x over previous
"""Trainium2 Bass kernel for nn_CorrelationLayer: B=16, C=256, H=W=128, max_disp=4.

out[b, (dy+4)*9+(dx+4), y, x] = sum_c f1n[b,c,y,x] * f2n[b,c,y-dy,x-dx]
with f1n/f2n L2-normalized over c (zero-fill outside the image).

Strategy (per core, 2 samples, pure batch data-parallel over 8 cores):
  Bands (8 rows) x patches (8x16 px). Per patch the PE computes a banded
  Gram G[128 px, 16 slots * 24 wx] against a 16-row f2 ring buffer
  (K=256 via 2-pass PSUM accumulation, f32r operands).
  DVE fuses normalization: data = G * rsqrt(|f1[px]|^2) * rsqrt(|f2[win]|^2)
  (row scalar per partition + column vector broadcast). rsqrt uses the raw
  ACT Rsqrt LUT (sole ACT table -> no per-band ACT_TABLE_LOAD thrash; LUT
  accuracy is well inside the 2e-2 budget).
  GPSIMD indirect_copy realigns window rows per pixel-row -> H[128, 9*24].
  PE transposes H via a truncated-bf16 bitcast view (1 cyc/row) into a
  per-band PSUM tile [108, 8p, 2, 128]; one big ACT + one big DVE copy
  drain it into bf16 band tiles; DMA stores to a padded bf16 output
  [216, H, W] (2KB runs). Host converts to f32 and unscrambles e -> dx,
  reversing both disp axes.

  Overlap structure: the two samples of a core are processed interleaved
  (independent rings) so each sample's f2 ring refill + norm chain hides
  under the other sample's compute; the patch loop is software-pipelined
  with lookahead 2 because engine queues are in-order (gram p+1/p+2 are
  enqueued before transpose p so the PE never waits on the DVE->GPSIMD
  realignment chain); f1 bands are DMA'd contiguously (4KB runs) and
  re-laid-out patch-major on ACT, since matmul stationary APs allow only
  one free dimension.
"""

import sys

sys.path.insert(0, "/opt/trn_rl_repo")

import numpy as np

B, C, H, W = 16, 256, 128, 128
NCORES = 8
SPC = B // NCORES        # samples per core
CHI = 2                  # c = chi*128 + partition
NB = 16                  # bands per image
RB = 8                   # rows per band
NP = 8                   # patches per band
PW = 16                  # patch width
WIN = 24                 # f2 window width per patch (16 + 8)
SLOTS = 16               # ring rows
RW = 136                 # ring width (128 + 2*4 zero pad)
NA = 9                   # row displacements (a = 4 - dy)
NEB = 3                  # 8-element blocks per window row
INNER = 8
NIDX = 28                # 27 used + 1 pad (alignment)
HFREE = NIDX * INNER     # 224
USED = NA * WIN          # 216
GN = SLOTS * WIN         # gram free size 384

_CACHE = {}


def _idx_table(parity: int) -> np.ndarray:
    t = np.zeros((128, 2), dtype=np.uint16)
    for r in range(RB):
        for j in range(NA * NEB):
            a, eb = divmod(j, NEB)
            slot = (8 * parity + r + a + 12) % SLOTS
            t[16 * r + (j % 16), j // 16] = slot * WIN + INNER * eb
    return t


def _build_nc():
    import concourse.tile as tile
    from concourse import bacc, mybir

    f32 = mybir.dt.float32
    f32r = mybir.dt.float32r
    bf16 = mybir.dt.bfloat16

    nc = bacc.Bacc("TRN2", target_bir_lowering=False, debug=False)

    f1d = nc.dram_tensor("feat1", [SPC, C, H, W], f32r, kind="ExternalInput").ap()
    f2d = nc.dram_tensor("feat2", [SPC, C, H, W], f32r, kind="ExternalInput").ap()
    idx_e = nc.dram_tensor("idx_e", [128, 2], mybir.dt.uint16, kind="ExternalInput").ap()
    idx_o = nc.dram_tensor("idx_o", [128, 2], mybir.dt.uint16, kind="ExternalInput").ap()
    ident = nc.dram_tensor("ident", [128, 128], bf16, kind="ExternalInput").ap()
    ones = nc.dram_tensor("ones", [128, 1], mybir.dt.float16, kind="ExternalInput").ap()
    zeros = nc.dram_tensor("zeros", [128, 1088], f32r, kind="ExternalInput").ap()
    opad = nc.dram_tensor("out_pad", [SPC, USED, H, W], bf16, kind="ExternalOutput").ap()

    f1v = f1d.rearrange("s (chi p) y x -> s p chi y x", p=128)
    f2v = f2d.rearrange("s (chi p) y x -> s p chi y x", p=128)

    def act_rsqrt(out, in_):
        eng = nc.scalar
        bias = nc.const_aps.scalar_like(0.0, in_)
        return eng.add_instruction(
            mybir.InstActivation(
                name=nc.get_next_instruction_name(),
                func=mybir.ActivationFunctionType.Rsqrt,
                ins=[
                    eng.lower_ap(in_),
                    eng.lower_ap(bias),
                    mybir.ImmediateValue(dtype=f32, value=1.0),
                    mybir.ImmediateValue(dtype=f32, value=0.0),
                ],
                outs=[eng.lower_ap(out)],
            )
        )

    def hi_bf16(ap):
        """Truncated-bf16 view of an f32 AP (high half of each word)."""
        n = ap.shape[-1]
        return ap.bitcast(bf16).rearrange(
            "p (n two) -> p n two", two=2
        )[:, :, 1]

    with tile.TileContext(nc) as tc:
        with (
            tc.tile_pool(name="persist", bufs=1) as pers,
            tc.tile_pool(name="f1p", bufs=2) as f1p,
            tc.tile_pool(name="f1pmp", bufs=2) as f1pmp,
            tc.tile_pool(name="sq1p", bufs=2) as sq1p,
            tc.tile_pool(name="sq2p", bufs=2) as sq2p,
            tc.tile_pool(name="s1p", bufs=2) as s1p,
            tc.tile_pool(name="datap", bufs=3) as datap,
            tc.tile_pool(name="hp", bufs=3) as hp,
            tc.tile_pool(name="htp", bufs=2) as htp,
            tc.tile_pool(name="gps", bufs=2, space="PSUM") as gps,
            tc.tile_pool(name="tpps", bufs=2, space="PSUM") as tpps,
            tc.tile_pool(name="s1ps", bufs=1, space="PSUM") as s1ps,
            tc.tile_pool(name="s2ps", bufs=1, space="PSUM") as s2ps,
        ):
            # persistent tiles (per-sample rings)
            ring = pers.tile([128, SPC, CHI, SLOTS, RW], f32r)   # f2 rows
            s2g = pers.tile([1, SPC, SLOTS, RW], f32)            # rsqrt(|f2 row|^2)
            s2gB = pers.tile([128, SPC, SLOTS, RW], f32)         # bcast over partitions
            isbe = pers.tile([128, 2], mybir.dt.uint16)
            isbo = pers.tile([128, 2], mybir.dt.uint16)
            idsb = pers.tile([128, 128], bf16)
            onesb = pers.tile([128, 1], mybir.dt.float16)

            nc.sync.dma_start(isbe[:], idx_e)
            nc.sync.dma_start(isbo[:], idx_o)
            nc.sync.dma_start(idsb[:], ident)
            nc.sync.dma_start(onesb[:], ones)

            # zero x-pad columns of the rings (once; loads never touch them)
            zv = zeros.rearrange("p (a b) -> p a b", b=4)
            for s in range(SPC):
                for chi in range(CHI):
                    nc.sync.dma_start(ring[:, s, chi, :, 0:4], zv[:, 0:SLOTS, :])
                    nc.sync.dma_start(ring[:, s, chi, :, 132:136], zv[:, 0:SLOTS, :])
            # 1.0 (not 0.0): s2gB is a divisor in the STT; pads/unloaded
            # slots must divide G=0 by a nonzero value (0/0 would NaN)
            nc.vector.memset(s2g[:], 1.0)
            nc.vector.memset(s2gB[:], 1.0)

            def load_f2_rows(s, rows):
                """DMA f2 rows (consecutive image rows) into ring slots, plus
                squares -> norms -> rsqrt -> broadcast. rows are 4-aligned."""
                i = 0
                while i < len(rows):
                    j = i
                    while (
                        j + 1 < len(rows)
                        and rows[j + 1] % SLOTS == (rows[j] % SLOTS) + 1
                    ):
                        j += 1
                    run = rows[i : j + 1]
                    s0 = run[0] % SLOTS
                    n = len(run)
                    for chi in range(CHI):
                        nc.sync.dma_start(
                            ring[:, s, chi, s0 : s0 + n, 4 : 4 + W],
                            f2v[s, :, chi, run[0] : run[0] + n, :],
                        )
                    i = j + 1
                # norms in 4-row chunks: rsqrt(ss) = exp(-0.5 * ln(ss))
                for i in range(0, len(rows), 4):
                    chunk = rows[i : i + 4]
                    s0 = chunk[0] % SLOTS
                    ncr = len(chunk)
                    sq2 = sq2p.tile([128, CHI, 4, W], mybir.dt.float16, tag="sq2")
                    nc.scalar.activation(
                        sq2[:, :, :ncr, :],
                        ring[:, s, :, s0 : s0 + ncr, 4 : 4 + W].bitcast(f32),
                        mybir.ActivationFunctionType.Square,
                    )
                    ss = s2ps.tile([1, 4 * W], f32, tag="s2ss")
                    for chi in range(CHI):
                        nc.tensor.matmul(
                            ss[:, : ncr * W],
                            onesb[:],
                            sq2[:, chi, :ncr, :],
                            start=(chi == 0),
                            stop=(chi == CHI - 1),
                        )
                    act_rsqrt(
                        s2g[0:1, s, s0 : s0 + ncr, 4 : 4 + W],
                        ss[:, : ncr * W].rearrange("p (a b) -> p a b", b=W),
                    )
                    nc.gpsimd.partition_broadcast(
                        s2gB[:, s, s0 : s0 + ncr, :],
                        s2g[0:1, s, s0 : s0 + ncr, :],
                    )

            def zero_slots(s, slots):
                s0, n = slots[0], len(slots)
                zv2 = zeros.rearrange("p (a b) -> p a b", b=RW)
                for chi in range(CHI):
                    nc.sync.dma_start(ring[:, s, chi, s0 : s0 + n, :], zv2[:, 0:n, :])

            for b in range(NB):
                y0 = RB * b
                isb = isbe if b % 2 == 0 else isbo
                for s in range(SPC):
                    # --- f2 ring maintenance: ensure rows [y0-4, y0+12) ---
                    if b == 0:
                        zero_slots(s, [12, 13, 14, 15])  # rows -4..-1
                        load_f2_rows(s, list(range(0, 12)))
                    elif b == NB - 1:
                        load_f2_rows(s, list(range(y0 + 4, H)))
                        zero_slots(s, [0, 1, 2, 3])      # rows 128..131
                    else:
                        load_f2_rows(s, list(range(y0 + 4, y0 + 12)))

                    # --- f1 band (contiguous rows) + norms ---
                    f1t = f1p.tile([128, CHI, RB, W], f32r, tag="f1")
                    for chi in range(CHI):
                        nc.sync.dma_start(
                            f1t[:, chi], f1v[s, :, chi, y0 : y0 + RB, :]
                        )
                    # patch-major views/tiles: matmul stationary APs need ONE
                    # free dim, so re-layout on ACT (cheap big-op copies,
                    # split per chi: ACT ISA patterns allow max 3 free dims)
                    f1pm = f1pmp.tile([128, CHI, NP, RB, PW], bf16, tag="f1pm")
                    sq1 = sq1p.tile(
                        [128, CHI, NP, RB, PW], mybir.dt.float16, tag="sq1"
                    )
                    for chi in range(CHI):
                        f1tv = f1t[:, chi].bitcast(f32).rearrange(
                            "c r (np cx) -> c np r cx", cx=PW
                        )
                        nc.scalar.copy(f1pm[:, chi], f1tv)
                        nc.scalar.activation(
                            sq1[:, chi], f1tv, mybir.ActivationFunctionType.Square
                        )
                    s1ss = s1ps.tile([128, NP], f32, tag="s1ss")
                    for p in range(NP):
                        for chi in range(CHI):
                            nc.tensor.matmul(
                                s1ss[:, p : p + 1],
                                sq1[:, chi, p].rearrange("c a b -> c (a b)"),
                                onesb[:],
                                start=(chi == 0),
                                stop=(chi == CHI - 1),
                            )
                    s1r = s1p.tile([128, NP], f32, tag="s1r")
                    act_rsqrt(s1r[:], s1ss[:])

                    # --- output band tiles (bf16) ---
                    ht0 = htp.tile([108, RB, NP, PW], bf16, tag="ht0")
                    ht1 = htp.tile([108, RB, NP, PW], bf16, tag="ht1")
                    tpb = tpps.tile([108, NP, 2, 128], bf16, tag="tpb")

                    # software-pipelined patch loop (lookahead 2): the PE
                    # queue is in-order, so emit gram(p+1), gram(p+2) BEFORE
                    # transpose(p) — the PE never waits on the DVE->GPSIMD
                    # chain that produces ht(p)
                    pend = []

                    def transp(args):
                        p, ht = args
                        htv = hi_bf16(ht[:])
                        nc.tensor.transpose(tpb[:, p, 0], htv[:, 0:108], idsb[:])
                        nc.tensor.transpose(tpb[:, p, 1], htv[:, 108:216], idsb[:])

                    for p in range(NP):
                        x0 = PW * p
                        # Gram (bf16: lhsT from re-layout, rhs hi-half view)
                        g = gps.tile([128, SLOTS, WIN], f32, tag="g")
                        for chi in range(CHI):
                            rw = ring[:, s, chi, :, x0 : x0 + WIN].bitcast(
                                bf16
                            ).rearrange("c sl (x two) -> c sl x two", two=2)[
                                :, :, :, 1
                            ]
                            nc.tensor.matmul(
                                g[:].rearrange("p s x -> p (s x)"),
                                f1pm[:, chi, p].rearrange("c a b -> c (a b)"),
                                rw,
                                start=(chi == 0),
                                stop=(chi == CHI - 1),
                            )
                        # fused normalization
                        data = datap.tile([128, GN], f32, tag="data")
                        nc.vector.scalar_tensor_tensor(
                            data[:].rearrange("p (a b) -> p a b", b=WIN),
                            g[:],
                            s1r[:, p : p + 1],
                            s2gB[:, s, :, x0 : x0 + WIN],
                            op0=mybir.AluOpType.mult,
                            op1=mybir.AluOpType.mult,
                        )
                        # realign window rows per pixel-row (shared idx per r)
                        ht = hp.tile([128, HFREE], f32, tag="h")
                        nc.gpsimd.indirect_copy(
                            ht[:].rearrange("p (a b) -> p a b", b=INNER),
                            data[:].rearrange("p (a b) -> p a b", b=INNER),
                            isb[:],
                            True,
                        )
                        pend.append((p, ht))
                        if len(pend) == 3:
                            transp(pend.pop(0))
                    while pend:
                        transp(pend.pop(0))

                    # batched PSUM drain: one big copy per half (ACT + DVE)
                    nc.scalar.copy(ht0[:], tpb[:, :, 0].rearrange(
                        "d p (r cx) -> d r p cx", cx=PW))
                    nc.vector.tensor_copy(ht1[:], tpb[:, :, 1].rearrange(
                        "d p (r cx) -> d r p cx", cx=PW))

                    opv = opad[s, :, y0 : y0 + RB, :].rearrange(
                        "d r (p cx) -> d r p cx", cx=PW
                    )
                    nc.scalar.dma_start(opv[0:108], ht0[:])
                    nc.scalar.dma_start(opv[108:216], ht1[:])

    nc.compile()
    return nc


def _constants():
    import ml_dtypes

    return {
        "idx_e": _idx_table(0),
        "idx_o": _idx_table(1),
        "ident": np.eye(128, dtype=ml_dtypes.bfloat16),
        "ones": np.ones((128, 1), dtype=np.float16),
        "zeros": np.zeros((128, 1088), dtype=np.float32),
    }


def _unscramble(pad: np.ndarray) -> np.ndarray:
    """pad [SPC, 216, H, W] -> out [SPC, 81, H, W].

    pad[s, a*24+e, y, x] = corr for dy = 4-a, dx = 4-(e - x%16)."""
    pad = np.asarray(pad).astype(np.float32)
    pad = pad.reshape(SPC, NA, WIN, H, W)
    tmp = np.empty((SPC, NA, NA, H, W), dtype=pad.dtype)
    for xm in range(PW):
        tmp[:, :, :, :, xm::PW] = pad[:, :, xm : xm + NA, :, xm::PW]
    # d order: (dy+4, dx+4) = (8-a, 8-adx)
    return tmp[:, ::-1, ::-1].reshape(SPC, NA * NA, H, W)


def kernel(feat1: np.ndarray, feat2: np.ndarray) -> np.ndarray:
    from concourse.bass_utils import run_bass_kernel_spmd

    feat1 = np.ascontiguousarray(np.asarray(feat1, dtype=np.float32))
    feat2 = np.ascontiguousarray(np.asarray(feat2, dtype=np.float32))

    if "nc" not in _CACHE:
        _CACHE["nc"] = _build_nc()
    nc = _CACHE["nc"]

    consts = _constants()
    in_maps = []
    for core in range(NCORES):
        s0 = SPC * core
        m = {"feat1": feat1[s0 : s0 + SPC], "feat2": feat2[s0 : s0 + SPC]}
        m.update(consts)
        in_maps.append(m)

    trace = _CACHE.get("trace", False)
    res = run_bass_kernel_spmd(nc, in_maps, list(range(NCORES)), trace=trace)
    _CACHE["last_results"] = res

    out = np.empty((B, NA * NA, H, W), dtype=np.float32)
    for core in range(NCORES):
        s0 = SPC * core
        out[s0 : s0 + SPC] = _unscramble(res.results[core]["out_pad"])
    return out
